# revision 10
# baseline (speedup 1.0000x reference)
"""Trainium2 Bass kernel for nn_DecoderWithAttention (B=100 captioning decoder:
per-step additive attention over R=15 regions, two LSTM cells, two V=10000
heads, NT=5 steps).

Strategy: 8-way tensor parallelism with the (padded) batch replicated via
per-step AllGathers; data-parallel preamble. Each core owns
  - a 128-row slice of both LSTMs' hidden dim (gate rows i/f/g/o),
  - a 1280-row slice of each vocab head,
  - a 512-wide slice of the image-feature dim (for the attended embedding),
  - 13 of the 104 (padded) samples for the preamble (autoencoder topics,
    attention keys att1, image mean).
Activations are feature-major (features on SBUF partitions, batch on the free
axis) so AllGather's partition-axis concat lands shards directly in matmul
layout. Weights are bf16 (fp32 PSUM accumulation); biases fold into ScalarE
activation evictions. The image-mean and last-word-embedding contributions to
LSTM1's gates are precomputed once. The loop is ordered so the TensorE always
has independent work (deferred preds head, early hh-parts) during the
AllGathers, keeping the PE HAM-warm.
"""

import sys

sys.path.insert(0, "/opt/trn_rl_repo")

import numpy as np
import ml_dtypes

bf16 = ml_dtypes.bfloat16

B, R, F = 100, 15, 4096
V, E, D, A, T = 10000, 1024, 1024, 512, 500
NT, L = 5, 52
NC = 8
BP = 104   # padded batch = 8 * 13
BS = 13    # per-core preamble batch shard
VP = 1280  # per-core vocab rows (padded vocab 10240)
FS = F // NC  # 512, per-core feature slice for awe

_STATE = {}


def _build_program():
    import concourse.mybir as mybir
    import concourse.tile as tile
    from concourse import bacc
    from concourse.ap import AP

    dt = mybir.dt
    AF = mybir.ActivationFunctionType
    OP = mybir.AluOpType
    X = mybir.AxisListType.X

    nc = bacc.Bacc("TRN2", target_bir_lowering=False, debug=False, num_devices=NC)

    def din(name, shape, dtype):
        return nc.dram_tensor(name, list(shape), dtype, kind="ExternalInput")

    bf = dt.bfloat16
    f32 = dt.float32

    # ---- external inputs (host-prepped per-core shards) ----
    img_dp = din("img_dp", (F, BS, R), bf)
    img_f = din("img_f", (FS, BP, R), bf)
    w_feat = din("w_feat", (F, A), bf)
    w_lt = din("w_lt", (F, 1024), bf)
    blt = din("blt", (128, 1024), f32)
    cwt = din("cwt", (120, 4, 6), bf)
    cbt = din("cbt", (128, 6), f32)
    w1h2 = din("w1h2", (1024, 512), bf)
    w1img = din("w1img", (F, 512), bf)
    w1emb = din("w1emb", (1024, 512), bf)
    whh1 = din("whh1", (1024, 512), bf)
    b1 = din("b1", (128, 512), f32)
    w2awe = din("w2awe", (F, 512), bf)
    w2h1 = din("w2h1", (1024, 512), bf)
    whh2 = din("whh2", (1024, 512), bf)
    b2 = din("b2", (128, 512), f32)
    wdec = din("wdec", (1024, 512), bf)
    wtop = din("wtop", (512, 512), bf)
    battn = din("battn", (128, 4), f32)
    wfull = din("wfull", (128, 4), bf)
    wfc1 = din("wfc1", (1024, VP), bf)
    bfc1 = din("bfc1", (128, VP), f32)
    wfc = din("wfc", (1024, VP), bf)
    bfc = din("bfc", (128, VP), f32)
    embsel = din("embsel", (1024, NT, BP), bf)
    ident = din("ident", (128, 128), bf)

    p1_out = nc.dram_tensor("p1_out", [NT, BP, VP], f32, kind="ExternalOutput")
    p_out = nc.dram_tensor("p_out", [NT, BP, VP], f32, kind="ExternalOutput")

    RG = [list(range(NC))]

    def ag(src_ap, dst_ap):
        nc.gpsimd.collective_compute(
            "AllGather", OP.bypass, replica_groups=RG,
            ins=[src_ap.opt()], outs=[dst_ap.opt()],
        )

    with tile.TileContext(nc) as tc:
      with (
          tc.tile_pool(name="keep", bufs=1) as keep,
          tc.tile_pool(name="kdram", bufs=1, space="DRAM") as kdram,
      ):
        # ---- long-lived SBUF state ----
        att1_sb = keep.tile([128, 4, BP, R], bf, name="att1_sb")
        topicsT_sb = keep.tile([128, 4, BP, NT], bf, name="topicsT_sb")
        imgF_sb = keep.tile([128, 4, BP, R], bf, name="imgF_sb")
        g1c_sb = keep.tile([128, NT, 512], f32, name="g1c_sb")
        h1T_sb = keep.tile([128, 8, BP], bf, name="h1T_sb")
        h2T_sb = keep.tile([128, 8, BP], bf, name="h2T_sb")
        c1_sb = keep.tile([128, 128], f32, name="c1_sb")
        c2_sb = keep.tile([128, 128], f32, name="c2_sb")
        b1_sb = keep.tile([128, 512], f32, name="b1_sb")
        b2_sb = keep.tile([128, 512], f32, name="b2_sb")
        battn_sb = keep.tile([128, 4], f32, name="battn_sb")
        bfc1_sb = keep.tile([128, VP], f32, name="bfc1_sb")
        bfc_sb = keep.tile([128, VP], f32, name="bfc_sb")
        wf_sb = keep.tile([128, 4], bf, name="wf_sb")
        ones_sb = keep.tile([1, 128], bf, name="ones_sb")
        ident_sb = keep.tile([128, 128], bf, name="ident_sb")

        nc.sync.dma_start(b1_sb[:], b1[:])
        nc.sync.dma_start(b2_sb[:], b2[:])
        nc.sync.dma_start(battn_sb[:], battn[:])
        nc.sync.dma_start(bfc1_sb[:], bfc1[:])
        nc.sync.dma_start(bfc_sb[:], bfc[:])
        nc.sync.dma_start(wf_sb[:], wfull[:])
        nc.sync.dma_start(ident_sb[:], ident[:])
        nc.scalar.dma_start(
            imgF_sb[:], img_f[:].rearrange("(ft p) b r -> p ft b r", p=128)
        )
        nc.gpsimd.memset(ones_sb[:], 1.0)
        nc.gpsimd.memset(h1T_sb[:], 0.0)
        nc.gpsimd.memset(h2T_sb[:], 0.0)
        nc.gpsimd.memset(c1_sb[:], 0.0)
        nc.gpsimd.memset(c2_sb[:], 0.0)
        nc.gpsimd.memset(topicsT_sb[:], 0.0)

        imgmean_sh = kdram.tile([F, BS], bf, name="imgmean_sh")
        imgmean_all = kdram.tile(
            [NC, F, BS], bf, name="imgmean_all", addr_space="Shared"
        )
        att1_sh = kdram.tile([A, BS * R], bf, name="att1_sh")
        att1_all = kdram.tile(
            [NC, A, BS * R], bf, name="att1_all", addr_space="Shared"
        )
        lin_poly = kdram.tile([2, BS * R, 512], bf, name="lin_poly")
        top_sh = kdram.tile([BS * 500, 6], bf, name="top_sh")
        top_all = kdram.tile(
            [NC, BS * 500, 6], bf, name="top_all", addr_space="Shared"
        )

        # ============ PREAMBLE (one overlapped phase) ============
        with (
            tc.tile_pool(name="pre", bufs=1) as pre,
            tc.tile_pool(name="pre_ps", bufs=2, space="PSUM") as pre_ps,
        ):
            img_dp_sb = pre.tile([128, 32, BS * R], bf, name="img_dp_sb")
            nc.sync.dma_start(
                img_dp_sb[:], img_dp[:].rearrange("(kt p) b r -> p kt (b r)", p=128)
            )
            w1emb_sb = pre.tile([128, 8, 512], bf, name="w1emb_sb")
            nc.sync.dma_start(
                w1emb_sb[:], w1emb[:].rearrange("(kt p) m -> p kt m", p=128)
            )
            embsel_sb = pre.tile([128, 8, NT, BP], bf, name="embsel_sb")
            nc.sync.dma_start(
                embsel_sb[:], embsel[:].rearrange("(kt p) t b -> p kt t b", p=128)
            )

            # 1) g1c emb part first (batch-major); b1 folded in here
            for t in range(NT):
                ps = pre_ps.tile([128, 512], f32, name="psE", tag="psP0", bufs=2)
                for kt in range(8):
                    nc.tensor.matmul(
                        ps[:BP, :],
                        embsel_sb[:, kt, t, :],
                        w1emb_sb[:, kt, :],
                        start=(kt == 0), stop=(kt == 7),
                    )
                nc.vector.tensor_tensor(
                    g1c_sb[:BP, t, :], ps[:BP, :], b1_sb[:BP, :], op=OP.add
                )

            # 2) img_mean (DP) -> AG early (overlaps with att1/lin below)
            imgmean_dp = pre.tile([128, 32, BS], bf, name="imgmean_dp")
            for kt in range(32):
                msum = pre.tile([128, BS], f32, name="msum", tag="msum", bufs=3)
                nc.vector.tensor_reduce(
                    msum[:],
                    img_dp_sb[:, kt, :].rearrange("p (b r) -> p b r", b=BS),
                    axis=X, op=OP.add,
                )
                nc.scalar.activation(
                    imgmean_dp[:, kt, :], msum[:], AF.Copy, scale=1.0 / 15.0
                )
            nc.sync.dma_start(
                imgmean_sh[:].rearrange("(kt p) b -> p kt b", p=128), imgmean_dp[:]
            )
            ag(imgmean_sh[:], imgmean_all[:])
            imgmean_sb = pre.tile([128, 32, BP], bf, name="imgmean_sb")
            for rk in range(NC):
                nc.sync.dma_start(
                    imgmean_sb[:, :, BS * rk : BS * (rk + 1)],
                    imgmean_all[rk].rearrange("(kt p) b -> p kt b", p=128),
                )

            # 3) att1 (DP): weight-chunk streamed
            att1_dp = pre.tile([128, 4, BS * R], bf, name="att1_dp")
            psA = [
                pre_ps.tile(
                    [128, BS * R], f32, name=f"psA{at}", tag=f"psP{at}", bufs=2
                )
                for at in range(4)
            ]
            for ck in range(4):
                wfch = pre.tile([128, 8, 512], bf, name="wfch", tag="wstr", bufs=3)
                nc.sync.dma_start(
                    wfch[:],
                    w_feat[1024 * ck : 1024 * (ck + 1), :].rearrange(
                        "(kt p) a -> p kt a", p=128
                    ),
                )
                for at in range(4):
                    for ktl in range(8):
                        nc.tensor.matmul(
                            psA[at][:],
                            wfch[:, ktl, 128 * at : 128 * (at + 1)],
                            img_dp_sb[:, 8 * ck + ktl, :],
                            start=(ck == 0 and ktl == 0),
                            stop=(ck == 3 and ktl == 7),
                        )
            for at in range(4):
                nc.scalar.copy(att1_dp[:, at, :], psA[at][:])
            nc.sync.dma_start(
                att1_sh[:].rearrange("(at p) x -> p at x", p=128), att1_dp[:]
            )
            ag(att1_sh[:], att1_all[:])
            for rk in range(NC):
                nc.sync.dma_start(
                    att1_sb[:, :, BS * rk : BS * (rk + 1), :],
                    att1_all[rk].rearrange("(at p) (b r) -> p at b r", p=128, b=BS),
                )

            # 4) lin (batch-major, wlt streamed) + polyphase split
            blt_sb = pre.tile([128, 1024], f32, name="blt_sb")
            nc.sync.dma_start(blt_sb[:], blt[:])
            lin_bm = pre.tile([128, 2, 1024], bf, name="lin_bm")
            psL = [
                pre_ps.tile([128, 512], f32, name=f"psL{i}", tag=f"psP{i}", bufs=2)
                for i in range(4)
            ]
            for ck in range(4):
                wlch = pre.tile([128, 8, 1024], bf, name="wlch", tag="wstr2", bufs=3)
                nc.sync.dma_start(
                    wlch[:],
                    w_lt[1024 * ck : 1024 * (ck + 1), :].rearrange(
                        "(kt p) m -> p kt m", p=128
                    ),
                )
                for Mt in range(2):
                    pn = 128 if Mt == 0 else BS * R - 128
                    for ch in range(2):
                        for ktl in range(8):
                            nc.tensor.matmul(
                                psL[2 * Mt + ch][:pn, :],
                                img_dp_sb[:, 8 * ck + ktl, 128 * Mt : 128 * Mt + pn],
                                wlch[:, ktl, 512 * ch : 512 * (ch + 1)],
                                start=(ck == 0 and ktl == 0),
                                stop=(ck == 3 and ktl == 7),
                            )
            for Mt in range(2):
                pn = 128 if Mt == 0 else BS * R - 128
                for ch in range(2):
                    nc.vector.tensor_tensor(
                        lin_bm[:pn, Mt, 512 * ch : 512 * (ch + 1)],
                        psL[2 * Mt + ch][:pn, :],
                        blt_sb[:pn, 512 * ch : 512 * (ch + 1)],
                        op=OP.add,
                    )
            lin_pq = pre.tile([128, 2, 2, 512], bf, name="lin_pq")
            for Mt in range(2):
                pn = 128 if Mt == 0 else BS * R - 128
                for q in range(2):
                    nc.vector.tensor_copy(
                        lin_pq[:pn, Mt, q, :], lin_bm[:pn, Mt, q::2]
                    )
                    nc.sync.dma_start(
                        lin_poly[q, 128 * Mt : 128 * Mt + pn, :],
                        lin_pq[:pn, Mt, q, :],
                    )

            # 5) conv topics (im2col from polyphase DRAM layout)
            cw_sb = pre.tile([120, 4, 6], bf, name="cw_sb")
            nc.sync.dma_start(cw_sb[:], cwt[:])
            cbt_sb = pre.tile([128, 6], f32, name="cbt_sb")
            nc.sync.dma_start(cbt_sb[:], cbt[:])
            lin_poly_h = lin_poly[:].tensor
            top_h = top_sh[:].tensor
            for h in range(2):
                imcol = pre.tile([120, 4, BS, 250], bf, name="imcol", tag="imcol")
                nc.gpsimd.memset(imcol[:, 3, :, :], 0.0)
                for pair in range(26):
                    q, v = pair // 13, pair % 13
                    kt, g = pair // 8, pair % 8
                    sap = AP(
                        lin_poly_h,
                        q * (BS * R * 512) + v + 250 * h,
                        [[512, 15], [R * 512, BS], [1, 250]],
                    )
                    nc.sync.dma_start(imcol[15 * g : 15 * g + 15, kt, :, :], sap)
                tbw = pre.tile([125, BS, 2, 6], bf, name="tbw", tag="tbw", bufs=2)
                for b in range(BS):
                    for qq in range(2):
                        psc = pre_ps.tile(
                            [125, 6], f32, name="psC", tag="psP0", bufs=2
                        )
                        for kt in range(4):
                            nc.tensor.matmul(
                                psc[:],
                                imcol[:, kt, b, 125 * qq : 125 * (qq + 1)],
                                cw_sb[:, kt, :],
                                start=(kt == 0), stop=(kt == 3),
                            )
                        nc.vector.tensor_tensor(
                            tbw[:, b, qq, :], psc[:], cbt_sb[:125, :], op=OP.add
                        )
                for qq in range(2):
                    dst = AP(
                        top_h,
                        (250 * h + 125 * qq) * 6,
                        [[6, 125], [500 * 6, BS], [1, 6]],
                    )
                    nc.sync.dma_start(dst, tbw[:, :, qq, :])
            ag(top_sh[:], top_all[:])
            topall_h = top_all[:].tensor
            for wt in range(4):
                pn = 128 if wt < 3 else 116
                for rk in range(NC):
                    nc.sync.dma_start(
                        topicsT_sb[:pn, wt, BS * rk : BS * (rk + 1), :],
                        AP(
                            topall_h,
                            rk * BS * 500 * 6 + 128 * wt * 6,
                            [[6, pn], [500 * 6, BS], [1, 5]],
                        ),
                    )

            # 6) g1c img part (needs imgmean AG) accumulated onto emb part
            g1img_sb = pre.tile([128, 512], f32, name="g1img_sb")
            psI = pre_ps.tile([128, 512], f32, name="psI", tag="psP1", bufs=2)
            for ck in range(4):
                wick = pre.tile([128, 8, 512], bf, name="wick", tag="wstr", bufs=3)
                nc.sync.dma_start(
                    wick[:],
                    w1img[1024 * ck : 1024 * (ck + 1), :].rearrange(
                        "(kt p) m -> p kt m", p=128
                    ),
                )
                for ktl in range(8):
                    nc.tensor.matmul(
                        psI[:BP, :],
                        imgmean_sb[:, 8 * ck + ktl, :],
                        wick[:, ktl, :],
                        start=(ck == 0 and ktl == 0),
                        stop=(ck == 3 and ktl == 7),
                    )
            nc.scalar.copy(g1img_sb[:BP, :], psI[:BP, :])
            for t in range(NT):
                nc.vector.tensor_tensor(
                    g1c_sb[:BP, t, :], g1c_sb[:BP, t, :],
                    g1img_sb[:BP, :], op=OP.add,
                )

        # ============ DECODE LOOP ============
        with (
            tc.tile_pool(name="wts", bufs=1) as wts,
            tc.tile_pool(name="wstream", bufs=3) as wstream,
            tc.tile_pool(name="work", bufs=1) as work,
            tc.tile_pool(name="ldram", bufs=2, space="DRAM") as ldram,
            tc.tile_pool(name="ps_m", bufs=4, space="PSUM") as ps_m,
            tc.tile_pool(name="ps_g", bufs=1, space="PSUM") as ps_g,
        ):
            def wload(name, src):
                t_ = wts.tile([128, 8, 512], bf, name=name)
                nc.sync.dma_start(
                    t_[:], src[:].rearrange("(kt p) m -> p kt m", p=128)
                )
                return t_

            wh2_sb = wload("wh2_sb", w1h2)
            whh1_sb = wload("whh1_sb", whh1)
            w2h1_sb = wload("w2h1_sb", w2h1)
            whh2_sb = wload("whh2_sb", whh2)
            wdec_sb = wload("wdec_sb", wdec)
            wtop_sb = wts.tile([128, 4, 512], bf, name="wtop_sb")
            nc.sync.dma_start(
                wtop_sb[:], wtop[:].rearrange("(kt p) m -> p kt m", p=128)
            )
            wfc1_sb = wts.tile([128, 8, VP], bf, name="wfc1_sb")
            nc.sync.dma_start(
                wfc1_sb[:], wfc1[:].rearrange("(kt p) m -> p kt m", p=128)
            )
            wfc_sb = wts.tile([128, 8, VP], bf, name="wfc_sb")
            nc.sync.dma_start(
                wfc_sb[:], wfc[:].rearrange("(kt p) m -> p kt m", p=128)
            )

            ACTF = {0: AF.Sigmoid, 1: AF.Sigmoid, 2: AF.Tanh, 3: AF.Sigmoid}

            def lstm_bm(psum, gadd, bname, c_sb, hb_out):
                """batch-major gates (BP,512) [i|f|g|o] -> c update -> hb (128,BP)"""
                gs = work.tile([128, 512], f32, name="gs" + bname, tag="gs" + bname)
                nc.vector.tensor_tensor(gs[:BP, :], psum[:BP, :], gadd, op=OP.add)
                ac = work.tile([128, 512], f32, name="ac" + bname, tag="ac" + bname)
                nc.scalar.activation(ac[:BP, 0:256], gs[:BP, 0:256], AF.Sigmoid)
                nc.scalar.activation(ac[:BP, 256:384], gs[:BP, 256:384], AF.Tanh)
                nc.scalar.activation(ac[:BP, 384:512], gs[:BP, 384:512], AF.Sigmoid)
                t1 = work.tile([128, 128], f32, name="t1", tag="pw1")
                nc.vector.tensor_tensor(
                    t1[:BP, :], ac[:BP, 0:128], ac[:BP, 256:384], op=OP.mult
                )
                t2 = work.tile([128, 128], f32, name="t2", tag="pw2")
                nc.vector.tensor_tensor(
                    t2[:BP, :], ac[:BP, 128:256], c_sb[:BP, :], op=OP.mult
                )
                nc.vector.tensor_tensor(c_sb[:BP, :], t1[:BP, :], t2[:BP, :], op=OP.add)
                tc_ = work.tile([128, 128], f32, name="tc_", tag="pw3")
                nc.scalar.activation(tc_[:BP, :], c_sb[:BP, :], AF.Tanh)
                hf = work.tile([128, 128], bf, name="hf", tag="pw4")
                nc.vector.tensor_tensor(
                    hf[:BP, :], ac[:BP, 384:512], tc_[:BP, :], op=OP.mult
                )
                pst = ps_m.tile([128, 512], bf, name="pst", tag="psm")
                nc.tensor.transpose(pst[:, :BP], hf[:BP, :], ident_sb[:BP, :BP])
                nc.vector.tensor_copy(hb_out[:], pst[:, :BP])

            def preds_head(t, w_sb, hT, bias_sb, out_dram, tag):
                """one vocab head, batch-major: h stationary, W moving (3 chunks)"""
                for c0, cw_ in ((0, 512), (512, 512), (1024, 256)):
                    ps = ps_m.tile([128, 512], f32, name="psp", tag="psm")
                    for kt in range(8):
                        nc.tensor.matmul(
                            ps[:BP, :cw_], hT[:, kt, :],
                            w_sb[:, kt, c0 : c0 + cw_],
                            start=(kt == 0), stop=(kt == 7),
                        )
                    stg = work.tile(
                        [128, 512], f32, name=f"stg{tag}", tag=f"stg{tag}", bufs=2
                    )
                    nc.vector.tensor_tensor(
                        stg[:BP, :cw_], ps[:BP, :cw_],
                        bias_sb[:BP, c0 : c0 + cw_], op=OP.add,
                    )
                    nc.scalar.dma_start(
                        out_dram[t, :, c0 : c0 + cw_], stg[:BP, :cw_]
                    )

            for t in range(NT):
                # ---- stream W_ih2[:, :4096] slice for this step ----
                wawe_chunks = []
                for ck in range(8):
                    wch = wstream.tile(
                        [128, 4, 512], bf, name="wch", tag="wawe", bufs=3
                    )
                    nc.sync.dma_start(
                        wch[:],
                        w2awe[512 * ck : 512 * (ck + 1), :].rearrange(
                            "(kt p) m -> p kt m", p=128
                        ),
                    )
                    wawe_chunks.append(wch)

                # ---- gates2 psum group opened early: hh2 part (uses h2(t-1))
                psg2 = ps_g.tile([128, 512], f32, name="psg2", tag="psg2")
                for kt in range(8):
                    nc.tensor.matmul(
                        psg2[:BP, :], h2T_sb[:, kt, :], whh2_sb[:, kt, :],
                        start=(kt == 0), stop=False,
                    )

                # ---- LSTM1 gates (batch-major) ----
                pg1 = ps_m.tile([128, 512], f32, name="pg1", tag="psm")
                for kt in range(8):
                    nc.tensor.matmul(
                        pg1[:BP, :], h2T_sb[:, kt, :], wh2_sb[:, kt, :],
                        start=(kt == 0), stop=False,
                    )
                for kt in range(8):
                    nc.tensor.matmul(
                        pg1[:BP, :], h1T_sb[:, kt, :], whh1_sb[:, kt, :],
                        start=False, stop=(kt == 7),
                    )
                h1b = work.tile([128, BP], bf, name="h1b", tag="h1b")
                lstm_bm(pg1, g1c_sb[:BP, t, :], "1", c1_sb, h1b)

                h1_sh = ldram.tile([128, BP], bf, name="h1_sh", tag="h1_sh")
                h1_all = ldram.tile(
                    [NC, 128, BP], bf, name="h1_all", tag="h1_all",
                    addr_space="Shared",
                )
                nc.sync.dma_start(h1_sh[:], h1b[:])
                ag(h1_sh[:], h1_all[:])

                # ---- deferred preds head from previous step (fills AG h1) ----
                if t > 0:
                    preds_head(t - 1, wfc_sb, h2T_sb, bfc_sb, p_out, "p")

                nc.sync.dma_start(
                    h1T_sb[:], h1_all[:].rearrange("rk p b -> p rk b")
                )

                # ---- attention dt = Wdec@h1 + Wtop@topic_t (+biases) ----
                dt_sb = work.tile([128, 4, BP], bf, name="dt_sb", tag="dt", bufs=1)
                for at in range(4):
                    ps = ps_m.tile([128, 512], f32, name="psd", tag="psm")
                    psv = ps[:, :BP]
                    for kt in range(8):
                        nc.tensor.matmul(
                            psv, wdec_sb[:, kt, 128 * at : 128 * (at + 1)],
                            h1T_sb[:, kt, :], start=(kt == 0), stop=False,
                        )
                    for kt in range(4):
                        nc.tensor.matmul(
                            psv, wtop_sb[:, kt, 128 * at : 128 * (at + 1)],
                            topicsT_sb[:, kt, :, t], start=False, stop=(kt == 3),
                        )
                    nc.scalar.activation(
                        dt_sb[:, at, :], psv, AF.Identity,
                        bias=battn_sb[:, at : at + 1],
                    )

                # gates2 h1 part (after h1T reload; fills attention phase)
                for kt in range(8):
                    nc.tensor.matmul(
                        psg2[:BP, :], h1T_sb[:, kt, :], w2h1_sb[:, kt, :],
                        start=False, stop=False,
                    )

                # att = relu(att1 + dt), one A-tile at a time; scores
                # accumulate over A-tiles into 4 held psum chunks
                chunks = [(c0, min(512, BP * R - c0)) for c0 in range(0, BP * R, 512)]
                pssl = [
                    ps_m.tile([128, 512], f32, name=f"pss{ci}", tag="psm")
                    for ci in range(len(chunks))
                ]
                for at in range(4):
                    am = work.tile([128, BP, R], bf, name="am", tag="attmp", bufs=2)
                    nc.vector.tensor_tensor(
                        am[:], att1_sb[:, at, :, :],
                        dt_sb[:, at, :].unsqueeze(-1).to_broadcast((128, BP, R)),
                        op=OP.add,
                    )
                    attc = work.tile(
                        [128, BP * R], bf, name="attc", tag="attc", bufs=2
                    )
                    nc.vector.tensor_scalar_max(
                        attc[:].rearrange("p (b r) -> p b r", b=BP), am[:], 0.0
                    )
                    for ci, (c0, cw_) in enumerate(chunks):
                        nc.tensor.matmul(
                            pssl[ci][:1, :cw_], wf_sb[:, at : at + 1],
                            attc[:, c0 : c0 + cw_],
                            start=(at == 0), stop=(at == 3),
                        )
                scoresf = work.tile(
                    [1, BP * R], bf, name="scoresf", tag="scoresf", bufs=1
                )
                for ci, (c0, cw_) in enumerate(chunks):
                    nc.scalar.copy(scoresf[:, c0 : c0 + cw_], pssl[ci][:1, :cw_])
                scores_bt = work.tile([BP, R], bf, name="scores_bt", tag="scbt")
                nc.sync.dma_start(scores_bt[:], scoresf[:])

                # ---- preds1 = W_fc1 @ h1 (fills PE during softmax/awe) ----
                preds_head(t, wfc1_sb, h1T_sb, bfc1_sb, p1_out, "p1")

                # softmax over R (batch-major)
                negmax = work.tile([BP, 1], f32, name="negmax", tag="negmax")
                nc.vector.tensor_reduce(
                    negmax[:], scores_bt[:], axis=X, op=OP.max, negate=True
                )
                esc = work.tile([BP, R], f32, name="esc", tag="esc")
                nc.scalar.activation(esc[:], scores_bt[:], AF.Exp, bias=negmax[:, :1])
                ssum = work.tile([BP, 1], f32, name="ssum", tag="ssum")
                nc.vector.tensor_reduce(ssum[:], esc[:], axis=X, op=OP.add)
                rinv = work.tile([BP, 1], f32, name="rinv", tag="rinv")
                nc.vector.reciprocal(rinv[:], ssum[:])
                alpha_bt = work.tile([BP, R], bf, name="alpha_bt", tag="alpha_bt")
                nc.vector.tensor_scalar_mul(alpha_bt[:], esc[:], rinv[:, :1])
                alpha_f = work.tile(
                    [1, BP * R], bf, name="alpha_f", tag="alpha_f", bufs=1
                )
                nc.sync.dma_start(alpha_f[:], alpha_bt[:])

                # broadcast alpha to 128 partitions via PE (ones^T @ alpha)
                alpha_bc = work.tile(
                    [128, BP * R], bf, name="alpha_bc", tag="alpha_bc", bufs=1
                )
                for c0 in range(0, BP * R, 512):
                    cw_ = min(512, BP * R - c0)
                    psb = ps_m.tile([128, 512], f32, name="psb", tag="psm")
                    nc.tensor.matmul(
                        psb[:, :cw_], ones_sb[:, :], alpha_f[:, c0 : c0 + cw_],
                        start=True, stop=True,
                    )
                    nc.vector.tensor_copy(alpha_bc[:, c0 : c0 + cw_], psb[:, :cw_])
                alpha_bcv = alpha_bc[:].rearrange("p (b r) -> p b r", b=BP)

                # awe (own 512-wide F slice) = sum_r imgF * alpha
                aweb = work.tile([128, 4, BP], bf, name="aweb", tag="aweb")
                for ft in range(4):
                    aw = work.tile([128, BP, R], bf, name="aw", tag="awtmp", bufs=1)
                    nc.vector.tensor_tensor(
                        aw[:], imgF_sb[:, ft, :, :], alpha_bcv, op=OP.mult
                    )
                    aws = work.tile([128, BP], f32, name="aws", tag="awsum", bufs=2)
                    nc.vector.tensor_reduce(aws[:], aw[:], axis=X, op=OP.add)
                    nc.vector.tensor_copy(aweb[:, ft, :], aws[:])
                awe_sh = ldram.tile([FS, BP], bf, name="awe_sh", tag="awe_sh")
                awe_all = ldram.tile(
                    [NC, FS, BP], bf, name="awe_all", tag="awe_all",
                    addr_space="Shared",
                )
                nc.sync.dma_start(
                    awe_sh[:].rearrange("(ft p) b -> p ft b", p=128), aweb[:]
                )
                ag(awe_sh[:], awe_all[:])
                aweT = work.tile([128, 32, BP], bf, name="aweT", tag="aweT", bufs=1)
                nc.sync.dma_start(
                    aweT[:], awe_all[:].rearrange("rk (ft p) b -> p (rk ft) b", p=128)
                )

                # ---- gates2 awe part + eviction + pointwise ----
                for ck in range(8):
                    for kt in range(4):
                        nc.tensor.matmul(
                            psg2[:BP, :], aweT[:, 4 * ck + kt, :],
                            wawe_chunks[ck][:, kt, :],
                            start=False, stop=(ck == 7 and kt == 3),
                        )
                h2b = work.tile([128, BP], bf, name="h2b", tag="h2b")
                lstm_bm(psg2, b2_sb[:BP, :], "2", c2_sb, h2b)

                h2_sh = ldram.tile([128, BP], bf, name="h2_sh", tag="h2_sh")
                h2_all = ldram.tile(
                    [NC, 128, BP], bf, name="h2_all", tag="h2_all",
                    addr_space="Shared",
                )
                nc.sync.dma_start(h2_sh[:], h2b[:])
                ag(h2_sh[:], h2_all[:])
                nc.sync.dma_start(
                    h2T_sb[:], h2_all[:].rearrange("rk p b -> p rk b")
                )

            # final deferred preds head (t = NT-1)
            preds_head(NT - 1, wfc_sb, h2T_sb, bfc_sb, p_out, "p")

    nc.compile()
    return nc


def _bfT(x):
    """fp32 -> bf16, transposed (contiguous)."""
    return np.ascontiguousarray(np.asarray(x, np.float32).T).astype(bf16)


def _prep_inputs(inputs):
    f = {k: np.asarray(v, np.float32) for k, v in inputs.items()
         if k not in ("encoded_captions", "caption_lengths")}
    caps = np.asarray(inputs["encoded_captions"])
    lens = np.asarray(inputs["caption_lengths"])

    img = f["image_features"]
    img_pad = np.zeros((BP, R, F), np.float32)
    img_pad[:B] = img
    imgT = np.ascontiguousarray(img_pad.transpose(2, 0, 1)).astype(bf16)  # (F,BP,R)

    lw = np.take_along_axis(caps, (lens - 1)[:, :, None].astype(np.int64), axis=2)[:, :, 0]
    lw_pad = np.zeros((BP, NT), np.int64)
    lw_pad[:B] = lw
    emb_rows = f["emb"][lw_pad.reshape(-1)].reshape(BP, NT, E)
    embsel = np.ascontiguousarray(emb_rows.transpose(2, 1, 0)).astype(bf16)  # (E,NT,BP)

    conv_w, conv_b = f["conv_w"], f["conv_b"]
    cwt = np.zeros((120, 4, 6), np.float32)
    for pair in range(26):
        q, v = pair // 13, pair % 13
        kt, g = pair // 8, pair % 8
        for i in range(15):
            cwt[15 * g + i, kt, :] = conv_w[:, 0, i, 2 * v + q]
    cwt = cwt.astype(bf16)
    cbt = np.tile(conv_b[None, :], (128, 1)).astype(np.float32)

    battn_v = f["b_feat"] + f["b_dec"] + f["b_top"]

    W_fc1_pad = np.zeros((VP * NC, D), np.float32)
    W_fc1_pad[:V] = f["W_fc1"]
    b_fc1_pad = np.zeros((VP * NC,), np.float32)
    b_fc1_pad[:V] = f["b_fc1"]
    W_fc_pad = np.zeros((VP * NC, D), np.float32)
    W_fc_pad[:V] = f["W_fc"]
    b_fc_pad = np.zeros((VP * NC,), np.float32)
    b_fc_pad[:V] = f["b_fc"]

    wtop_pad = np.zeros((512, A), np.float32)
    wtop_pad[:500] = f["W_top"].T
    wtop_pad = wtop_pad.astype(bf16)

    shared = {
        "w_feat": _bfT(f["W_feat"]),
        "w_lt": _bfT(f["W_lt"]),
        "blt": np.tile(f["b_lt"][None, :], (128, 1)).astype(np.float32),
        "cwt": cwt,
        "cbt": cbt,
        "wdec": _bfT(f["W_dec"]),
        "wtop": np.ascontiguousarray(wtop_pad),
        "battn": np.ascontiguousarray(battn_v.reshape(4, 128).T, dtype=np.float32),
        "wfull": np.ascontiguousarray(f["W_full"][0].reshape(4, 128).T).astype(bf16),
        "embsel": embsel,
        "ident": np.eye(128, dtype=np.float32).astype(bf16),
    }

    b1v = f["b_ih1"] + f["b_hh1"]
    b2v = f["b_ih2"] + f["b_hh2"]

    in_maps = []
    for k in range(NC):
        rows = np.concatenate(
            [np.arange(g * D + 128 * k, g * D + 128 * (k + 1)) for g in range(4)]
        )
        m = dict(shared)
        m["img_dp"] = np.ascontiguousarray(imgT[:, BS * k : BS * (k + 1), :])
        m["img_f"] = np.ascontiguousarray(imgT[FS * k : FS * (k + 1), :, :])
        m["w1h2"] = _bfT(f["W_ih1"][rows, :1024])
        m["w1img"] = _bfT(f["W_ih1"][rows, 1024:5120])
        m["w1emb"] = _bfT(f["W_ih1"][rows, 5120:])
        m["whh1"] = _bfT(f["W_hh1"][rows])
        m["b1"] = np.tile(b1v[rows][None, :], (128, 1)).astype(np.float32)
        m["w2awe"] = _bfT(f["W_ih2"][rows, :4096])
        m["w2h1"] = _bfT(f["W_ih2"][rows, 4096:])
        m["whh2"] = _bfT(f["W_hh2"][rows])
        m["b2"] = np.tile(b2v[rows][None, :], (128, 1)).astype(np.float32)
        vsl = slice(VP * k, VP * (k + 1))
        m["wfc1"] = _bfT(W_fc1_pad[vsl])
        m["bfc1"] = np.tile(b_fc1_pad[vsl][None, :], (128, 1)).astype(np.float32)
        m["wfc"] = _bfT(W_fc_pad[vsl])
        m["bfc"] = np.tile(b_fc_pad[vsl][None, :], (128, 1)).astype(np.float32)
        in_maps.append(m)
    return in_maps, caps, lens


def kernel(**inputs):
    from concourse import bass_utils

    if "nc" not in _STATE:
        _STATE["nc"] = _build_program()
    nc = _STATE["nc"]

    in_maps, caps, lens = _prep_inputs(inputs)
    res = bass_utils.run_bass_kernel_spmd(nc, in_maps, core_ids=list(range(NC)))

    p1 = np.concatenate([res.results[k]["p1_out"] for k in range(NC)], axis=2)
    p = np.concatenate([res.results[k]["p_out"] for k in range(NC)], axis=2)
    preds1 = np.ascontiguousarray(p1[:, :B, :V].transpose(1, 0, 2))
    preds = np.ascontiguousarray(p[:, :B, :V].transpose(1, 0, 2))
    return preds, preds1, caps, lens - 1
